# revision 1
# baseline (speedup 1.0000x reference)
"""Bass/Tile SPMD kernel for nn_GATModel: GAT(2-layer) + BiLSTM + bilinear.

8 cores: core c -> (unit u = c%4, half = c//4).
units: 0=lstm1_fwd(p) 1=lstm1_bwd(p) 2=lstm2_fwd(h) 3=lstm2_bwd(h)
Each core: 16 samples of its graph, full GAT, xproj for its unit,
512-step recurrence (leading no-op pad for bwd), AllGather(4), bilinear.
"""
import numpy as np
import ml_dtypes
import concourse.bass as bass
import concourse.mybir as mybir
from concourse import bacc
from concourse.tile import TileContext

F32 = mybir.dt.float32
F32R = mybir.dt.float32r
BF16 = mybir.dt.bfloat16
AF = mybir.ActivationFunctionType
AL = mybir.AluOpType
AX = mybir.AxisListType

L, S, H, HH, DEP, NL = 256, 255, 768, 384, 81, 3
N = L + S          # 511
NP = 512           # padded seq
KH = H // 128      # 6 chunks of feature dim
K2 = 2 * KH        # 12 chunks of 2H
G4 = 4 * HH        # 1536 gate width


def build_nc(nsamp=16, nstep=NP, debug=()):
    nc = bacc.Bacc()
    I = lambda name, shape, dt=BF16: nc.dram_tensor(name, shape, dt, kind="ExternalInput")
    xT   = I("xT",   [nsamp, 128, KH, L])
    spc  = I("spc",  [nsamp, 128, 2, 3], F32)
    spr  = I("spr",  [nsamp, 3, S], F32)
    embT = I("embT", [128, KH, DEP])
    W1   = I("W1",   [2, 128, KH, H])
    W1T  = I("W1T",  [128, 2, KH, H])
    a1   = I("a1",   [128, 2, 2, KH])
    W2   = I("W2",   [128, K2, H])
    W2T  = I("W2T",  [128, KH, 2 * H])
    a2   = I("a2",   [128, 2, KH])
    Iw   = I("Iw",   [128, 2, N])
    Id   = I("Id",   [128, 2, N])
    Jw   = I("Jw",   [128, 2, NP])
    Jd   = I("Jd",   [128, 2, NP])
    ones = I("ones", [1, NP])
    Wihb = I("Wihb", [128, KH + 1, G4])
    Whh  = I("Whh",  [128, 3, G4], F32)
    flags= I("flags",[128, 2], F32)
    bilW = I("bilW", [NL, 128, KH, H], F32)
    bilb = I("bilb", [nsamp, NL], F32)

    out = nc.dram_tensor("out", [nsamp, NL], F32, kind="ExternalOutput")
    dbg = {}
    if "gatT" in debug:
        dbg["gatT"] = nc.dram_tensor("dbg_gatT", [KH * 128, NP], F32, kind="ExternalOutput")
    if "h1T" in debug:
        dbg["h1T"] = nc.dram_tensor("dbg_h1T", [K2 * 128, N], F32, kind="ExternalOutput")
    if "z" in debug:
        dbg["z"] = nc.dram_tensor("dbg_z", [2 * 128, 770], F32, kind="ExternalOutput")
    if "zd" in debug:
        dbg["zd"] = nc.dram_tensor("dbg_zd", [2 * 128, 770], F32, kind="ExternalOutput")
    if "hfin" in debug:
        dbg["hfin"] = nc.dram_tensor("dbg_hfin", [128, 48], F32, kind="ExternalOutput")
    if "xp" in debug:
        dbg["xp"] = nc.dram_tensor("dbg_xp", [NP, G4], F32, kind="ExternalOutput")

    xproj = nc.dram_tensor("xproj", [NP, nsamp, G4], F32R)
    cc_in = nc.dram_tensor("cc_in", [3 * 128, nsamp], F32)
    cc_out = nc.dram_tensor("cc_out", [12 * 128, nsamp], F32)

    with TileContext(nc) as tc:
        _emit(nc, tc, locals(), nsamp, nstep, dbg)
    nc.finalize()
    return nc


def _emit(nc, tc, T, nsamp, nstep, dbg):
    xT, spc, spr, embT, W1, W1T, a1, W2, W2T, a2 = (
        T["xT"], T["spc"], T["spr"], T["embT"], T["W1"], T["W1T"], T["a1"],
        T["W2"], T["W2T"], T["a2"])
    Iw, Id, Jw, Jd, ones, Wihb, Whh, flags, bilW, bilb = (
        T["Iw"], T["Id"], T["Jw"], T["Jd"], T["ones"], T["Wihb"], T["Whh"],
        T["flags"], T["bilW"], T["bilb"])
    out, xproj, cc_in, cc_out = T["out"], T["xproj"], T["cc_in"], T["cc_out"]

    def ts_(eng, o, i, s1, s2, o0, o1=None):
        if o1 is None:
            return eng.tensor_scalar(o, i, s1, s2, op0=o0)
        return eng.tensor_scalar(o, i, s1, s2, op0=o0, op1=o1)

    with tc.tile_pool(name="wper", bufs=1) as wper, \
         tc.tile_pool(name="glob", bufs=1) as glob:
        # ---- persistent weight tiles ----
        w1e = []
        for h in range(2):
            t = wper.tile([128, KH, 770], BF16, tag=f"w1e{h}")
            nc.sync.dma_start(t[:, :, 0:H], W1[h])
            w1e.append(t)
        w2e = wper.tile([128, K2, 770], BF16)
        nc.sync.dma_start(w2e[:, :, 0:H], W2[:])
        wihb = wper.tile([128, KH + 1, G4], BF16)
        nc.sync.dma_start(wihb[:], Wihb[:])
        iwt = wper.tile([128, 2, N], BF16); nc.sync.dma_start(iwt[:], Iw[:])
        idt = wper.tile([128, 2, N], BF16); nc.sync.dma_start(idt[:], Id[:])
        jwt = wper.tile([128, 2, NP], BF16); nc.sync.dma_start(jwt[:], Jw[:])
        jdt = wper.tile([128, 2, NP], BF16); nc.sync.dma_start(jdt[:], Jd[:])
        onr = wper.tile([1, NP], BF16); nc.sync.dma_start(onr[:], ones[:])
        flg = wper.tile([128, 2], F32); nc.sync.dma_start(flg[:], flags[:])

        io0 = glob.tile([128, 1], F32)
        nc.gpsimd.iota(io0[:], [[1, 1]], channel_multiplier=1, allow_small_or_imprecise_dtypes=True)
        io1 = glob.tile([128, 1], F32)
        nc.gpsimd.iota(io1[:], [[1, 1]], base=128, channel_multiplier=1, allow_small_or_imprecise_dtypes=True)
        iorow = glob.tile([1, L], F32)
        nc.gpsimd.iota(iorow[:], [[1, L]], channel_multiplier=0, allow_small_or_imprecise_dtypes=True)
        iob = glob.tile([128, L], F32)
        nc.gpsimd.partition_broadcast(iob[:], iorow[:])
        ident = glob.tile([128, 128], F32)
        i16r = glob.tile([16, 16], F32R)
        ts_(nc.vector, ident[:], iob[:, 0:128], io0[:], None, AL.is_equal)
        nc.vector.tensor_copy(i16r[:], ident[0:16, 0:16])

        with tc.tile_pool(name="psmall", bufs=2, space="PSUM") as psmall, \
             tc.tile_pool(name="wprep", bufs=1) as wprep:
            a1t = wprep.tile([128, 2, 2, KH], BF16); nc.sync.dma_start(a1t[:], a1[:])
            a2t = wprep.tile([128, 2, KH], BF16); nc.sync.dma_start(a2t[:], a2[:])
            w1tt = wprep.tile([128, 2, KH, H], BF16); nc.sync.dma_start(w1tt[:], W1T[:])
            w2tt = wprep.tile([128, KH, 2 * H], BF16); nc.sync.dma_start(w2tt[:], W2T[:])
            embt = wprep.tile([128, KH, DEP], BF16); nc.sync.dma_start(embt[:], embT[:])
            wrow = wprep.tile([1, 2 * H], BF16)
            wt_dram = nc.dram_tensor(f"wt_scratch_{nc.next_id()}", [2 * H], BF16)
            for h in range(2):
                for lr in range(2):
                    wps = psmall.tile([1, 512], F32, tag="wt")
                    for n0, nw in ((0, 512), (512, 256)):
                        for k in range(KH):
                            nc.tensor.matmul(wps[:, 0:nw], a1t[:, h, lr, k].unsqueeze(1),
                                             w1tt[:, h, k, n0:n0 + nw], start=(k == 0), stop=(k == KH - 1))
                        nc.scalar.copy(wrow[:, n0:n0 + nw], wps[:, 0:nw])
                    nc.sync.dma_start(wt_dram[0:H].unsqueeze(0), wrow[:, 0:H])
                    nc.sync.dma_start(w1e[h][:, :, 768 + lr],
                                      wt_dram[0:H].rearrange("(c p) -> p c", p=128))
            for lr in range(2):
                for n0 in range(0, 2 * H, 512):
                    wps2 = psmall.tile([1, 512], F32, tag="wt")
                    for k in range(KH):
                        nc.tensor.matmul(wps2[:, :], a2t[:, lr, k].unsqueeze(1),
                                         w2tt[:, k, n0:n0 + 512], start=(k == 0), stop=(k == KH - 1))
                    nc.scalar.copy(wrow[:, n0:n0 + 512], wps2[:, :])
                nc.sync.dma_start(wt_dram[:].unsqueeze(0), wrow[:])
                nc.sync.dma_start(w2e[:, :, 768 + lr],
                                  wt_dram[:].rearrange("(c p) -> p c", p=128))
            ztab = []
            for h in range(2):
                zps = psmall.tile([DEP, 1024], F32, tag="ztab")
                zt = glob.tile([DEP, 770], BF16, tag=f"ztab{h}")
                for n0, nw in ((0, 512), (512, 258)):
                    for k in range(KH):
                        nc.tensor.matmul(zps[:, n0:n0 + nw], embt[:, k, :],
                                         w1e[h][:, k, n0:n0 + nw], start=(k == 0), stop=(k == KH - 1))
                nc.scalar.copy(zt[:], zps[:, 0:770])
                ztab.append(zt)

        # ================= per-sample GAT =================
        with tc.tile_pool(name="samp", bufs=1) as samp, \
             tc.tile_pool(name="attn", bufs=1) as attn, \
             tc.tile_pool(name="feat", bufs=2) as feat, \
             tc.tile_pool(name="fet1", bufs=1) as fet1, \
             tc.tile_pool(name="big", bufs=1) as big, \
             tc.tile_pool(name="pz", bufs=1, space="PSUM") as pz, \
             tc.tile_pool(name="pagg", bufs=3, space="PSUM") as pagg, \
             tc.tile_pool(name="palt", bufs=1, space="PSUM") as palt, \
             tc.tile_pool(name="pvec", bufs=1, space="PSUM") as pvec:

            for s in range(nsamp):
                xts = samp.tile([128, KH, L], BF16, tag="xts")
                nc.sync.dma_start(xts[:], xT[s])
                spct = samp.tile([128, 2, 3], F32, tag="spct")
                nc.sync.dma_start(spct[:], spc[s])
                sprt = samp.tile([1, 3, S], F32, tag="sprt")
                nc.sync.dma_start(sprt[:], spr[s])

                # packed scratch: cols 0:255 negw0 | 256:511 negw1 | 512:768 negd0
                # | 768:1024 negd1 | 1024:1279 w0b | 1280:1535 w1b | 1536:1792 scratch ma
                # | 1792:2048 mb
                SC = samp.tile([128, 2048], F32, tag="scr")
                negw = [SC[:, 0:S], SC[:, 256:256 + S]]
                negd = [SC[:, 512:768], SC[:, 768:1024]]
                w0b, w1b = SC[:, 1024:1024 + S], SC[:, 1280:1280 + S]
                nc.gpsimd.partition_broadcast(w0b, sprt[:, 0, :])
                nc.gpsimd.partition_broadcast(w1b, sprt[:, 1, :])
                for mi, iot in ((0, io0), (1, io1)):
                    ma, mb = SC[:, 1536:1536 + S], SC[:, 1792:1792 + S]
                    ts_(nc.vector, ma, w0b, iot[:], None, AL.is_equal)
                    ts_(nc.vector, mb, w1b, iot[:], None, AL.is_equal)
                    nc.vector.tensor_max(ma, ma, mb)
                    ts_(nc.vector, negw[mi], ma, 1e9, 1e9, AL.mult, AL.subtract)
                for mi, rows in ((0, 128), (1, 127)):
                    ma, mb = SC[0:rows, 1536:1536 + L], SC[0:rows, 1792:1792 + L]
                    ts_(nc.vector, ma, iob[0:rows], spct[0:rows, mi, 0:1], None, AL.is_equal)
                    ts_(nc.vector, mb, iob[0:rows], spct[0:rows, mi, 1:2], None, AL.is_equal)
                    nc.vector.tensor_max(ma, ma, mb)
                    ts_(nc.vector, negd[mi][0:rows], ma, 1e9, 1e9, AL.mult, AL.subtract)
                labb = samp.tile([DEP, S], F32, tag="labb")
                nc.gpsimd.partition_broadcast(labb[:], sprt[:, 2, :], channels=DEP)
                eh = samp.tile([DEP, S], BF16, tag="eh")
                ts_(nc.vector, eh[:], labb[:], io0[0:DEP], None, AL.is_equal)

                # attention scratch (per sample): bcast rows + small cols
                SB = samp.tile([128, 1344], F32, tag="scrb")
                eldb, elwb = SB[:, 0:512], SB[:, 512:1024]
                

                def vrow_transpose(col_tiles, widths, dstap):
                    w = sum(widths)
                    rps = pvec.tile([1, 512], F32, tag="vrow")
                    off = 0
                    for ct, cw in zip(col_tiles, widths):
                        nc.tensor.matmul(rps[:, off:off + cw], ct, ident[0:cw, 0:cw],
                                         is_transpose=True, start=True, stop=True)
                        off += cw
                    row = SB[0:1, 1024:1024 + w]
                    nc.scalar.copy(row, rps[:, 0:w])
                    nc.gpsimd.partition_broadcast(dstap[:, 0:w], row)

                scn = [0]
                def attn_block(er_cols, el_bc, neg_tiles, rows_l, src_n, alpk):
                    als = []
                    for i, rows in enumerate(rows_l):
                        xb = attn.tile([128, 512], F32, tag="ax")
                        nc.vector.scalar_tensor_tensor(
                            xb[0:rows, 0:src_n], el_bc[0:rows, 0:src_n], er_cols[i],
                            neg_tiles[i][0:rows, 0:src_n], op0=AL.add, op1=AL.add)
                        nc.vector.scalar_tensor_tensor(
                            xb[0:rows, 0:src_n], xb[0:rows, 0:src_n], 0.2,
                            xb[0:rows, 0:src_n], op0=AL.mult, op1=AL.max)
                        cb = 1280 + 8 * ((scn[0]) % 8); scn[0] += 1
                        nmx = SB[0:rows, cb:cb + 1]
                        nc.vector.tensor_reduce(nmx, xb[0:rows, 0:src_n], AX.X, AL.max, negate=True)
                        p = attn.tile([128, 512], F32, tag="ap")
                        ssum = SB[0:rows, cb + 1:cb + 2]
                        nc.scalar.activation(p[0:rows, 0:src_n], xb[0:rows, 0:src_n], AF.Exp,
                                             bias=nmx, scale=1.0, accum_out=ssum)
                        r = SB[0:rows, cb + 2:cb + 3]
                        nc.vector.reciprocal(r, ssum)
                        gate = SB[0:rows, cb + 3:cb + 4]
                        ts_(nc.vector, gate, nmx, 1e7, None, AL.is_lt)
                        rg = SB[0:rows, cb + 4:cb + 5]
                        nc.vector.tensor_mul(rg, r, gate)
                        al = alpk[:, i, :]
                        ts_(nc.vector, al[0:rows, 0:src_n], p[0:rows, 0:src_n], rg, None, AL.mult)
                        als.append(al)
                    return als

                def alT_mm(al_tiles, dst_rows_l, src_n, place, width, tag):
                    src_tiles = []
                    n_src_t = (src_n + 127) // 128
                    for mi in range(n_src_t):
                        mw = min(128, src_n - 128 * mi)
                        ps = palt.tile([128, width], F32, tag=tag)
                        for ki, dr in enumerate(dst_rows_l):
                            nc.tensor.matmul(ps[0:mw, :], al_tiles[ki][0:dr, 128 * mi:128 * mi + mw],
                                             place[0:dr, ki, :], start=(ki == 0), stop=(ki == len(dst_rows_l) - 1))
                        sb = feat.tile([128, width], BF16, tag=tag + "s")
                        nc.scalar.copy(sb[0:mw, :], ps[0:mw, :])
                        src_tiles.append(sb)
                    return src_tiles

                # ---- layer 1 ----
                h1T = big.tile([128, K2, N], BF16, tag="h1T")
                for h in range(2):
                    colt = fet1.tile([128, 32], F32, tag="cols")
                    zsb, erw_c, elw_c = [], [], []
                    for m in range(2):
                        zps = pz.tile([128, 1024], F32, tag="z")
                        for n0, nw in ((0, 512), (512, 258)):
                            for k in range(KH):
                                nc.tensor.matmul(zps[:, n0:n0 + nw],
                                                 xts[:, k, 128 * m:128 * (m + 1)],
                                                 w1e[h][:, k, n0:n0 + nw],
                                                 start=(k == 0), stop=(k == KH - 1))
                        zb = feat.tile([128, H], BF16, tag="zw")
                        nc.scalar.copy(zb[:], zps[:, 0:H])
                        ec = colt[:, 16 * h + 2 * m:16 * h + 2 * m + 2]
                        nc.vector.tensor_copy(ec, zps[:, 768:770])
                        zsb.append(zb); elw_c.append(ec[:, 0:1]); erw_c.append(ec[:, 1:2])
                        if "z" in dbg and s == nsamp - 1 and h == 0:
                            zf = fet1.tile([128, 1536], F32, tag="dbgcp")
                            nc.vector.tensor_copy(zf[:, 0:770], zps[:, 0:770])
                            nc.sync.dma_start(dbg["z"][128 * m:128 * (m + 1), :], zf[:, 0:770])
                    zdsb, erd_c, eld_c = [], [], []
                    for m, rows in ((0, 128), (1, 127)):
                        zps = pz.tile([128, 1024], F32, tag="z")
                        for n0, nw in ((0, 512), (512, 258)):
                            nc.tensor.matmul(zps[0:rows, n0:n0 + nw],
                                             eh[:, 128 * m:128 * m + rows],
                                             ztab[h][:, n0:n0 + nw], start=True, stop=True)
                        zb = feat.tile([128, H], BF16, tag="zd")
                        nc.scalar.copy(zb[0:rows], zps[0:rows, 0:H])
                        ec = colt[:, 16 * h + 4 + 2 * m:16 * h + 4 + 2 * m + 2]
                        nc.vector.tensor_copy(ec[0:rows], zps[0:rows, 768:770])
                        zdsb.append(zb); eld_c.append(ec[0:rows, 0:1]); erd_c.append(ec[0:rows, 1:2])
                        if "zd" in dbg and s == nsamp - 1 and h == 0:
                            zf = fet1.tile([128, 1536], F32, tag="dbgcp")
                            nc.vector.tensor_copy(zf[0:rows, 0:770], zps[0:rows, 0:770])
                            nc.sync.dma_start(dbg["zd"][128 * m:128 * m + rows, :], zf[0:rows, 0:770])
                    vrow_transpose(eld_c, [128, 127], eldb)
                    vrow_transpose(elw_c, [128, 128], elwb)
                    alpW = attn.tile([128, 2, 256], BF16, tag="alw")
                    alpD = attn.tile([128, 2, 256], BF16, tag="ald")
                    alW = attn_block(erw_c, eldb, negw, [128, 128], S, alpW)
                    alD = attn_block(erd_c, elwb, negd, [128, 127], L, alpD)
                    aTW = alT_mm(alW, [128, 128], S, iwt, N, "atw")
                    aTD = alT_mm(alD, [128, 127], L, idt, N, "atd")
                    for wave in range(2):
                        pss = []
                        for mt in range(3 * wave, 3 * wave + 3):
                            ps = pagg.tile([128, N], F32, tag="agg")
                            first = True
                            for ks, rows in ((0, 128), (1, 127)):
                                nc.tensor.matmul(ps[:], zdsb[ks][0:rows, 128 * mt:128 * (mt + 1)],
                                                 aTW[ks][0:rows, :], start=first, stop=False)
                                first = False
                            for ks in (0, 1):
                                nc.tensor.matmul(ps[:], zsb[ks][:, 128 * mt:128 * (mt + 1)],
                                                 aTD[ks][:, :], start=False, stop=(ks == 1))
                            pss.append((mt, ps))
                        for mt, ps in pss:
                            ex = fet1.tile([128, N], F32, tag="elux")
                            nc.scalar.activation(ex[:], ps[:], AF.Exp)
                            ts_(nc.vector, ex[:], ex[:], 1.0, -1.0, AL.min, AL.add)
                            nc.vector.scalar_tensor_tensor(
                                h1T[:, KH * h + mt, :], ps[:], 0.0, ex[:], op0=AL.max, op1=AL.add)
                if "h1T" in dbg and s == nsamp - 1:
                    for c in range(K2):
                        hf = fet1.tile([128, 1536], F32, tag="dbgcp")
                        nc.vector.tensor_copy(hf[:, 0:N], h1T[:, c, :])
                        nc.sync.dma_start(dbg["h1T"][128 * c:128 * (c + 1), :], hf[:, 0:N])

                # ---- layer 2 ----
                colt2 = fet1.tile([128, 16], F32, tag="cols2")
                z2sb, er2_c, el2_c = [], [], []
                for m, rows in ((0, 128), (1, 128), (2, 128), (3, 127)):
                    zps = pz.tile([128, 1024], F32, tag="z")
                    for n0, nw in ((0, 512), (512, 258)):
                        for k in range(K2):
                            nc.tensor.matmul(zps[0:rows, n0:n0 + nw],
                                             h1T[:, k, 128 * m:128 * m + rows],
                                             w2e[:, k, n0:n0 + nw],
                                             start=(k == 0), stop=(k == K2 - 1))
                    zb = fet1.tile([128, H], BF16, tag=f"z2{m}")
                    nc.scalar.copy(zb[0:rows], zps[0:rows, 0:H])
                    ec = colt2[:, 2 * m:2 * m + 2]
                    nc.vector.tensor_copy(ec[0:rows], zps[0:rows, 768:770])
                    z2sb.append(zb); el2_c.append(ec[0:rows, 0:1]); er2_c.append(ec[0:rows, 1:2])
                vrow_transpose(el2_c[2:], [128, 127], eldb)
                vrow_transpose(el2_c[:2], [128, 128], elwb)
                alpW2 = attn.tile([128, 2, 256], BF16, tag="alw")
                alpD2 = attn.tile([128, 2, 256], BF16, tag="ald")
                alW2 = attn_block(er2_c[:2], eldb, negw, [128, 128], S, alpW2)
                alD2 = attn_block(er2_c[2:], elwb, negd, [128, 127], L, alpD2)
                aTW2 = alT_mm(alW2, [128, 128], S, jwt, NP, "atw")
                aTD2 = alT_mm(alD2, [128, 127], L, jdt, NP, "atd")
                gatT = big.tile([128, KH, NP], BF16, tag="gatT")
                for wave in range(2):
                    pss = []
                    for mt in range(3 * wave, 3 * wave + 3):
                        ps = pagg.tile([128, NP], F32, tag="agg")
                        first = True
                        for ks, rows in ((0, 128), (1, 127)):
                            nc.tensor.matmul(ps[:], z2sb[2 + ks][0:rows, 128 * mt:128 * (mt + 1)],
                                             aTW2[ks][0:rows, :], start=first, stop=False)
                            first = False
                        for ks in (0, 1):
                            nc.tensor.matmul(ps[:], z2sb[ks][:, 128 * mt:128 * (mt + 1)],
                                             aTD2[ks][:, :], start=False, stop=(ks == 1))
                        pss.append((mt, ps))
                    for mt, ps in pss:
                        nc.scalar.copy(gatT[:, mt, :], ps[:])
                if "gatT" in dbg and s == nsamp - 1:
                    for c in range(KH):
                        gf = fet1.tile([128, 1536], F32, tag="dbgcp")
                        nc.vector.tensor_copy(gf[:, 0:NP], gatT[:, c, :])
                        nc.sync.dma_start(dbg["gatT"][128 * c:128 * (c + 1), :], gf[:, 0:NP])

                # ---- xproj ----
                for m in range(4):
                    pss = []
                    for ni in range(3):
                        ps = pagg.tile([128, 512], F32, tag="agg")
                        for k in range(KH):
                            nc.tensor.matmul(ps[:], gatT[:, k, 128 * m:128 * (m + 1)],
                                             wihb[:, k, 512 * ni:512 * (ni + 1)],
                                             start=(k == 0), stop=False)
                        nc.tensor.matmul(ps[:], onr[:, 128 * m:128 * (m + 1)],
                                         wihb[0:1, KH, 512 * ni:512 * (ni + 1)],
                                         start=False, stop=True)
                        pss.append((ni, ps))
                    xsb = big.tile([128, G4], F32R, tag="xsb")
                    for ni, ps in pss:
                        nc.scalar.copy(xsb[:, 512 * ni:512 * (ni + 1)], ps[:])
                    nc.sync.dma_start(xproj[128 * m:128 * (m + 1), s:s + 1, :], xsb[:].unsqueeze(1))

        if "xp" in dbg:
            with tc.tile_pool(name="xdbg", bufs=2) as xdbg:
                for m in range(4):
                    t = xdbg.tile([128, G4], F32R, tag="x")
                    nc.sync.dma_start(t[:].unsqueeze(1), xproj[128 * m:128 * (m + 1), nsamp - 1:nsamp, :])
                    t2 = xdbg.tile([128, G4], F32, tag="xf")
                    nc.vector.tensor_copy(t2[:], t[:])
                    nc.sync.dma_start(dbg["xp"][128 * m:128 * (m + 1), :], t2[:])

        # ================= recurrence =================
        with tc.tile_pool(name="rx", bufs=6) as rx, \
             tc.tile_pool(name="rst", bufs=2) as rst, \
             tc.tile_pool(name="rg", bufs=2) as rg, \
             tc.tile_pool(name="pg", bufs=1, space="PSUM") as pgp, \
             tc.tile_pool(name="ptr", bufs=2, space="PSUM") as ptr, \
             tc.tile_pool(name="rfin", bufs=1) as rfin:
            whhf = rfin.tile([128, 3, G4], F32)
            nc.sync.dma_start(whhf[:], Whh[:])
            whhr = rfin.tile([128, 3, G4], F32R)
            for k in range(3):
                nc.vector.tensor_copy(whhr[:, k, :], whhf[:, k, :])
            W3 = 3 * nsamp
            hT = rst.tile([128, W3], F32R, tag="hT")
            zed = rfin.tile([128, W3], F32)
            nc.vector.memset(zed[:], 0.0)
            nc.vector.tensor_copy(hT[:], zed[:])
            cst = rst.tile([nsamp, HH], F32, tag="c")
            nc.vector.memset(cst[:], 0.0)
            snapA = rfin.tile([128, W3], F32)
            snapB = rfin.tile([128, W3], F32)
            for t in range(nstep):
                xr = rx.tile([nsamp, G4], F32R, tag="xr")
                nc.sync.dma_start(xr[:], xproj[t])
                gps = {}
                for ci in (3, 0, 1, 2):
                    ps = pgp.tile([nsamp, HH], F32, tag=f"g{ci}")
                    nc.tensor.matmul(ps[:], i16r[0:nsamp, 0:nsamp], xr[:, HH * ci:HH * (ci + 1)],
                                     start=True, stop=False)
                    for kc in range(3):
                        nc.tensor.matmul(ps[:], hT[:, nsamp * kc:nsamp * (kc + 1)],
                                         whhr[:, kc, HH * ci:HH * (ci + 1)],
                                         start=False, stop=(kc == 2))
                    gps[ci] = ps
                so = rg.tile([nsamp, HH], F32, tag="so")
                nc.scalar.activation(so[:], gps[3][:], AF.Sigmoid)
                si = rg.tile([nsamp, HH], F32, tag="si")
                nc.scalar.activation(si[:], gps[0][:], AF.Sigmoid)
                sf = rg.tile([nsamp, HH], F32, tag="sf")
                nc.scalar.activation(sf[:], gps[1][:], AF.Sigmoid)
                tg = rg.tile([nsamp, HH], F32, tag="tg")
                nc.scalar.activation(tg[:], gps[2][:], AF.Tanh)
                t1 = rg.tile([nsamp, HH], F32, tag="t1")
                nc.vector.tensor_mul(t1[:], sf[:], cst[:])
                t2 = rg.tile([nsamp, HH], F32, tag="t2")
                nc.vector.tensor_mul(t2[:], si[:], tg[:])
                cn = rst.tile([nsamp, HH], F32, tag="c")
                nc.vector.tensor_add(cn[:], t1[:], t2[:])
                th = rg.tile([nsamp, HH], F32, tag="th")
                nc.scalar.activation(th[:], cn[:], AF.Tanh)
                hh = rg.tile([nsamp, HH], F32, tag="hh")
                nc.vector.tensor_mul(hh[:], so[:], th[:])
                tps = ptr.tile([128, W3], F32, tag="tr")
                for kc in range(3):
                    nc.tensor.matmul(tps[:, nsamp * kc:nsamp * (kc + 1)],
                                     hh[:, 128 * kc:128 * (kc + 1)], ident[0:nsamp, 0:nsamp],
                                     is_transpose=True, start=True, stop=True)
                hTn = rst.tile([128, W3], F32R, tag="hT")
                nc.vector.tensor_copy(hTn[:], tps[:])
                hT = hTn
                cst = cn
                if t == nstep - 2:
                    nc.vector.tensor_copy(snapA[:], tps[:])
                if t == nstep - 1:
                    nc.vector.tensor_copy(snapB[:], tps[:])
            hsel = rfin.tile([128, W3], F32)
            ts_(nc.vector, hsel[:], snapA[:], flg[:, 0:1], None, AL.mult)
            nc.vector.scalar_tensor_tensor(hsel[:], snapB[:], flg[:, 1:2], hsel[:],
                                           op0=AL.mult, op1=AL.add)
            if "hfin" in dbg:
                nc.sync.dma_start(dbg["hfin"][:, 0:W3], hsel[:])
            nc.sync.dma_start(cc_in[:].rearrange("(c p) f -> p c f", p=128),
                              hsel[:].rearrange("p (c f) -> p c f", c=3))

        # ================= collective + bilinear =================
        with tc.tile_pool(name="bil", bufs=1) as bil, \
             tc.tile_pool(name="pbil", bufs=1, space="PSUM") as pbil:
            nc.gpsimd.collective_compute(
                "AllGather", AL.bypass,
                replica_groups=[[0, 1, 2, 3], [4, 5, 6, 7]],
                ins=[cc_in.ap().opt()], outs=[cc_out.ap().opt()])
            gath = bil.tile([128, 12, nsamp], F32)
            nc.sync.dma_start(gath[:], cc_out[:].rearrange("(c p) f -> p c f", p=128))
            pT = bil.tile([128, KH, nsamp], F32R)
            nc.vector.tensor_copy(pT[:], gath[:, 0:KH, :])
            hps = pbil.tile([nsamp, H], F32, tag="htr")
            for kc in range(KH):
                nc.tensor.matmul(hps[:, 128 * kc:128 * (kc + 1)],
                                 gath[:, KH + kc, :], ident[:],
                                 is_transpose=True, start=True, stop=True)
            hsb = bil.tile([nsamp, H], F32)
            nc.scalar.copy(hsb[:], hps[:])
            bwsb = bil.tile([128, KH, H], F32)
            bwr = bil.tile([128, KH, H], F32R)
            outc = bil.tile([nsamp, NL], F32)
            junk = bil.tile([nsamp, H], F32)
            for k in range(NL):
                nc.sync.dma_start(bwsb[:], bilW[k])
                for c in range(KH):
                    nc.vector.tensor_copy(bwr[:, c, :], bwsb[:, c, :])
                vps = pbil.tile([nsamp, H], F32, tag="v")
                for n0 in (0, 512):
                    nw = min(512, H - n0)
                    for kc in range(KH):
                        nc.tensor.matmul(vps[:, n0:n0 + nw], pT[:, kc, :],
                                         bwr[:, kc, n0:n0 + nw],
                                         start=(kc == 0), stop=(kc == KH - 1))
                nc.vector.scalar_tensor_tensor(junk[:], vps[:], 1.0, hsb[:],
                                               op0=AL.mult, op1=AL.mult,
                                               accum_out=outc[:, k:k + 1])
            bbt = bil.tile([nsamp, NL], F32)
            nc.sync.dma_start(bbt[:], bilb[:])
            outt = bil.tile([nsamp, NL], F32)
            nc.vector.tensor_add(outt[:], outc[:], bbt[:])
            nc.sync.dma_start(out[:], outt[:])


# ===================== host-side preparation =====================
def _chunkP(a):
    """[X*128, ...] -> [128, X, ...] with p inner: out[p, c, ...] = a[128c+p]"""
    x = a.reshape(a.shape[0] // 128, 128, *a.shape[1:])
    return np.swapaxes(x, 0, 1)

def _bf(a):
    return np.ascontiguousarray(a.astype(ml_dtypes.bfloat16))

def _f(a):
    return np.ascontiguousarray(np.asarray(a, np.float32))

def make_in_maps(inp, nsamp=16):
    B = np.asarray(inp["prem_hidden_states"]).shape[0]
    emb = _f(inp["depend_emb"])
    in_maps = []
    embT_ = _bf(_chunkP(emb.T))                       # [128, KH, DEP]
    onehot = lambda idx, w: np.eye(w, dtype=np.float32)[idx]  # rows
    # placement mats (constant)
    wid = np.arange(256)
    Iw_ = _bf(_chunkP(onehot(wid, N)))                # Iw[p,2ch? -> [128,2,N]
    sid = np.arange(255)
    Id_rows = np.zeros((256, N), np.float32); Id_rows[:255] = onehot(256 + sid, N)
    Id_ = _bf(_chunkP(Id_rows))
    for c in range(8):
        unit, half = c % 4, c // 4
        g = "prem" if unit < 2 else "hypo"
        fwd = (unit % 2 == 0)
        lstm = "lstm1" if unit < 2 else "lstm2"
        dirn = "f" if fwd else "b"
        sl = slice(16 * half, 16 * half + nsamp)
        hid = _f(inp[f"{g}_hidden_states"])[sl]       # [nsamp, L, H]
        spans = np.asarray(inp[f"{g}_span"])[sl]      # [nsamp, S, 3]
        m = {}
        m["xT"] = _bf(np.stack([_chunkP(hid[s].T) for s in range(nsamp)]))
        spc_ = np.zeros((nsamp, 256, 3), np.float32)
        spc_[:, :255] = spans.astype(np.float32)
        m["spc"] = _f(np.stack([_chunkP(spc_[s]) for s in range(nsamp)]))
        m["spr"] = _f(np.swapaxes(spans.astype(np.float32), 1, 2))
        m["embT"] = embT_
        W1 = _f(inp[f"{g}_W1"])                       # [2,H,H]
        m["W1"] = _bf(np.stack([_chunkP(W1[h]) for h in range(2)]))
        m["W1T"] = _bf(np.stack([_chunkP(W1[h].T) for h in range(2)], axis=1))
        a1 = _f(inp[f"{g}_a1"])                       # [2, 2H]
        m["a1"] = _bf(np.stack(
            [np.stack([_chunkP(a1[h, lr * H:(lr + 1) * H, None])[:, :, 0] for lr in range(2)])
             for h in range(2)]).transpose(2, 0, 1, 3))   # -> [128,2,2,KH]
        W2 = _f(inp[f"{g}_W2"])                       # [2H, H]
        m["W2"] = _bf(_chunkP(W2))
        m["W2T"] = _bf(_chunkP(W2.T))
        a2 = _f(inp[f"{g}_a2"])                       # [2H]
        m["a2"] = _bf(np.stack([_chunkP(a2[lr * H:(lr + 1) * H, None])[:, :, 0] for lr in range(2)])
                      .transpose(1, 0, 2))            # [128,2,KH]
        m["Iw"] = Iw_; m["Id"] = Id_
        if fwd:
            Jw_r = onehot(wid, NP)
            Jd_rows = np.zeros((256, NP), np.float32); Jd_rows[:255] = onehot(256 + sid, NP)
            ones_ = np.ones((1, NP), np.float32); ones_[0, N] = 0.0
        else:
            Jw_r = onehot(511 - wid, NP)
            Jd_rows = np.zeros((256, NP), np.float32); Jd_rows[:255] = onehot(255 - sid, NP)
            ones_ = np.ones((1, NP), np.float32); ones_[0, 0] = 0.0
        m["Jw"] = _bf(_chunkP(Jw_r)); m["Jd"] = _bf(_chunkP(Jd_rows))
        m["ones"] = _bf(ones_)
        Wih = _f(inp[f"{lstm}_Wih_{dirn}"])           # [4HH, H]
        bb = _f(inp[f"{lstm}_b_{dirn}"])              # [4HH]
        Wihb_ = np.zeros((896, G4), np.float32)
        Wihb_[:H] = Wih.T
        Wihb_[H] = bb
        m["Wihb"] = _bf(_chunkP(Wihb_))               # [128, 7, G4]
        Whh_ = _f(inp[f"{lstm}_Whh_{dirn}"])          # [4HH, HH]
        m["Whh"] = _f(_chunkP(Whh_.T))                # [128, 3, G4]
        fl = np.zeros((128, 2), np.float32)
        fl[:, 0] = 1.0 if fwd else 0.0
        fl[:, 1] = 0.0 if fwd else 1.0
        m["flags"] = fl
        bilW = _f(inp["bil_W"])                       # [3,H,H]
        m["bilW"] = _f(np.stack([_chunkP(bilW[k]) for k in range(NL)]))
        m["bilb"] = _f(np.broadcast_to(_f(inp["bil_b"])[None, :], (nsamp, NL)).copy())
        in_maps.append(m)
    return in_maps


# ===================== harness entry point =====================
_NC_CACHE = {}

def _get_nc(nsamp=16, nstep=NP):
    key = (nsamp, nstep)
    if key not in _NC_CACHE:
        _NC_CACHE[key] = build_nc(nsamp=nsamp, nstep=nstep)
    return _NC_CACHE[key]


def kernel(**inputs):
    """Full-input entry: shards across 8 NeuronCores, runs the Bass kernel,
    returns the full [32, 3] float32 output."""
    inputs = {k: np.asarray(v) for k, v in inputs.items()}
    nc = _get_nc()
    in_maps = make_in_maps(inputs, nsamp=16)
    from concourse import bass_utils
    res = bass_utils.run_bass_kernel_spmd(nc, in_maps, core_ids=list(range(8)))
    out = np.concatenate([res.results[0]["out"], res.results[4]["out"]], 0)
    return out.astype(np.float32)



# revision 10
# speedup vs baseline: 2.1842x; 2.1842x over previous
"""Bass/Tile SPMD kernel for nn_GATModel: GAT(2-layer) + BiLSTM + bilinear.

8 cores: core c -> (graph = prem if c<4 else hypo, quarter q = c%4).
Each core: 8 samples of its graph, full GAT, xproj for BOTH LSTM
directions, then a hardware-looped 511-step recurrence running fwd+bwd
interleaved. Final hidden [8, 768] per core -> host-side bilinear.
"""
import numpy as np
import ml_dtypes
import concourse.bass as bass
import concourse.mybir as mybir
from concourse import bacc
from concourse.bass import ds
from concourse.tile import TileContext

F32 = mybir.dt.float32
F32R = mybir.dt.float32r
BF16 = mybir.dt.bfloat16
AF = mybir.ActivationFunctionType
AL = mybir.AluOpType
AX = mybir.AxisListType

L, S, H, HH, DEP, NL = 256, 255, 768, 384, 81, 3
N = L + S          # 511
NP = 512           # padded node count
KH = H // 128      # 6 chunks of feature dim
K2 = 2 * KH        # 12 chunks of 2H
G4 = 4 * HH        # 1536 gate width
G3 = 3 * HH        # 1152 sigmoid block (gate order i,f,o | g)
NSAMP = 8          # samples per core


def declare_tensors(nc, nsamp=NSAMP):
    I = lambda name, shape, dt=BF16: nc.dram_tensor(name, shape, dt, kind="ExternalInput")
    T = {}
    T["xT"]   = I("xT",   [nsamp, 128, KH, L])
    T["spc"]  = I("spc",  [nsamp, 128, 2, 3], F32)
    T["spr"]  = I("spr",  [nsamp, 3, S], F32)
    T["embT"] = I("embT", [128, KH, DEP])
    T["W1"]   = I("W1",   [2, 128, KH, H])
    T["W1T"]  = I("W1T",  [128, 2, KH, H])
    T["a1"]   = I("a1",   [128, 2, 2, KH])
    T["W2"]   = I("W2",   [128, K2, H])
    T["W2T"]  = I("W2T",  [128, KH, 2 * H])
    T["a2"]   = I("a2",   [128, 2, KH])
    T["Iw"]   = I("Iw",   [128, 2, N])
    T["Id"]   = I("Id",   [128, 2, N])
    T["Jw"]   = I("Jw",   [128, 2, NP])
    T["Jd"]   = I("Jd",   [128, 2, NP])
    T["ones"] = I("ones", [1, NP])
    T["Wihb"] = I("Wihb", [128, 2, KH + 1, G4])
    T["Whh"]  = I("Whh",  [128, 2, 3, G4], F32R)
    T["out"]  = nc.dram_tensor("out", [nsamp, 2 * HH], F32, kind="ExternalOutput")
    return T


def build_nc(nsamp=NSAMP, nstep=N):
    nc = bacc.Bacc()
    T = declare_tensors(nc, nsamp)
    with TileContext(nc) as tc:
        _emit(nc, tc, T, nsamp, nstep)
    nc.finalize()
    return nc


def _emit(nc, tc, T, nsamp, nstep, dbg=None):
    xT, spc, spr, embT, W1, W1T, a1, W2, W2T, a2 = (
        T["xT"], T["spc"], T["spr"], T["embT"], T["W1"], T["W1T"], T["a1"],
        T["W2"], T["W2T"], T["a2"])
    Iw, Id, Jw, Jd, ones, Wihb, Whh = (
        T["Iw"], T["Id"], T["Jw"], T["Jd"], T["ones"], T["Wihb"], T["Whh"])
    out = T["out"]

    def ts_(eng, o, i, s1, s2, o0, o1=None):
        if o1 is None:
            return eng.tensor_scalar(o, i, s1, s2, op0=o0)
        return eng.tensor_scalar(o, i, s1, s2, op0=o0, op1=o1)

    from contextlib import ExitStack
    with tc.tile_pool(name="glob", bufs=1) as glob, \
         tc.tile_pool(name="xpd", bufs=1, space="DRAM") as xpd, \
         ExitStack() as gat_stack:
        # DRAM scratch via a tile pool so Tile tracks the GAT-phase writes ->
        # recurrence reads dependency (raw dram_tensor accesses are untracked).
        xp2 = xpd.tile([2, NP, nsamp, G4], F32R)
        wper = gat_stack.enter_context(tc.tile_pool(name="wper", bufs=1))
        # ---- weight tiles (GAT phase only; freed before the recurrence) ----
        w1e = []
        for h in range(2):
            t = wper.tile([128, KH, 770], BF16, tag=f"w1e{h}")
            nc.sync.dma_start(t[:, :, 0:H], W1[h])
            w1e.append(t)
        w2e = wper.tile([128, K2, 770], BF16)
        nc.sync.dma_start(w2e[:, :, 0:H], W2[:])
        wihb = wper.tile([128, 2, KH + 1, G4], BF16)
        nc.sync.dma_start(wihb[:], Wihb[:])
        iwt = wper.tile([128, 2, N], BF16); nc.sync.dma_start(iwt[:], Iw[:])
        idt = wper.tile([128, 2, N], BF16); nc.sync.dma_start(idt[:], Id[:])
        jwt = wper.tile([128, 2, NP], BF16); nc.sync.dma_start(jwt[:], Jw[:])
        jdt = wper.tile([128, 2, NP], BF16); nc.sync.dma_start(jdt[:], Jd[:])
        onr = wper.tile([1, NP], BF16); nc.sync.dma_start(onr[:], ones[:])

        io0 = glob.tile([128, 1], F32)
        nc.gpsimd.iota(io0[:], [[1, 1]], channel_multiplier=1, allow_small_or_imprecise_dtypes=True)
        io1 = glob.tile([128, 1], F32)
        nc.gpsimd.iota(io1[:], [[1, 1]], base=128, channel_multiplier=1, allow_small_or_imprecise_dtypes=True)
        iorow = glob.tile([1, L], F32)
        nc.gpsimd.iota(iorow[:], [[1, L]], channel_multiplier=0, allow_small_or_imprecise_dtypes=True)
        iob = glob.tile([128, L], F32)
        nc.gpsimd.partition_broadcast(iob[:], iorow[:])
        ident = glob.tile([128, 128], F32)
        i8r = glob.tile([nsamp, nsamp], F32R)
        ts_(nc.vector, ident[:], iob[:, 0:128], io0[:], None, AL.is_equal)
        nc.vector.tensor_copy(i8r[:], ident[0:nsamp, 0:nsamp])

        with tc.tile_pool(name="psmall", bufs=2, space="PSUM") as psmall, \
             tc.tile_pool(name="wprep", bufs=1) as wprep:
            a1t = wprep.tile([128, 2, 2, KH], BF16); nc.sync.dma_start(a1t[:], a1[:])
            a2t = wprep.tile([128, 2, KH], BF16); nc.sync.dma_start(a2t[:], a2[:])
            w1tt = wprep.tile([128, 2, KH, H], BF16); nc.sync.dma_start(w1tt[:], W1T[:])
            w2tt = wprep.tile([128, KH, 2 * H], BF16); nc.sync.dma_start(w2tt[:], W2T[:])
            embt = wprep.tile([128, KH, DEP], BF16); nc.sync.dma_start(embt[:], embT[:])
            wrow = wprep.tile([1, 2 * H], BF16)
            wt_dram = nc.dram_tensor(f"wt_scratch_{nc.next_id()}", [2 * H], BF16)
            for h in range(2):
                for lr in range(2):
                    wps = psmall.tile([1, 512], F32, tag="wt")
                    for n0, nw in ((0, 512), (512, 256)):
                        for k in range(KH):
                            nc.tensor.matmul(wps[:, 0:nw], a1t[:, h, lr, k].unsqueeze(1),
                                             w1tt[:, h, k, n0:n0 + nw], start=(k == 0), stop=(k == KH - 1))
                        nc.scalar.copy(wrow[:, n0:n0 + nw], wps[:, 0:nw])
                    nc.sync.dma_start(wt_dram[0:H].unsqueeze(0), wrow[:, 0:H])
                    nc.sync.dma_start(w1e[h][:, :, 768 + lr],
                                      wt_dram[0:H].rearrange("(c p) -> p c", p=128))
            for lr in range(2):
                for n0 in range(0, 2 * H, 512):
                    wps2 = psmall.tile([1, 512], F32, tag="wt")
                    for k in range(KH):
                        nc.tensor.matmul(wps2[:, :], a2t[:, lr, k].unsqueeze(1),
                                         w2tt[:, k, n0:n0 + 512], start=(k == 0), stop=(k == KH - 1))
                    nc.scalar.copy(wrow[:, n0:n0 + 512], wps2[:, :])
                nc.sync.dma_start(wt_dram[:].unsqueeze(0), wrow[:])
                nc.sync.dma_start(w2e[:, :, 768 + lr],
                                  wt_dram[:].rearrange("(c p) -> p c", p=128))
            ztab = []
            for h in range(2):
                zps = psmall.tile([DEP, 1024], F32, tag="ztab")
                zt = glob.tile([DEP, 770], BF16, tag=f"ztab{h}")
                for n0, nw in ((0, 512), (512, 258)):
                    for k in range(KH):
                        nc.tensor.matmul(zps[:, n0:n0 + nw], embt[:, k, :],
                                         w1e[h][:, k, n0:n0 + nw], start=(k == 0), stop=(k == KH - 1))
                nc.scalar.copy(zt[:], zps[:, 0:770])
                ztab.append(zt)

        # ================= per-sample GAT =================
        with tc.tile_pool(name="samp", bufs=1) as samp, \
             tc.tile_pool(name="attn", bufs=1) as attn, \
             tc.tile_pool(name="feat", bufs=2) as feat, \
             tc.tile_pool(name="fet1", bufs=1) as fet1, \
             tc.tile_pool(name="big", bufs=1) as big, \
             tc.tile_pool(name="pz", bufs=1, space="PSUM") as pz, \
             tc.tile_pool(name="pagg", bufs=3, space="PSUM") as pagg, \
             tc.tile_pool(name="palt", bufs=1, space="PSUM") as palt, \
             tc.tile_pool(name="pvec", bufs=1, space="PSUM") as pvec:

            for s in range(nsamp):
                xts = samp.tile([128, KH, L], BF16, tag="xts")
                nc.sync.dma_start(xts[:], xT[s])
                spct = samp.tile([128, 2, 3], F32, tag="spct")
                nc.sync.dma_start(spct[:], spc[s])
                sprt = samp.tile([1, 3, S], F32, tag="sprt")
                nc.sync.dma_start(sprt[:], spr[s])

                # packed scratch: cols 0:255 negw0 | 256:511 negw1 | 512:768 negd0
                # | 768:1024 negd1 | 1024:1279 w0b | 1280:1535 w1b | 1536:1792 scratch ma
                # | 1792:2048 mb
                SC = samp.tile([128, 2048], F32, tag="scr")
                negw = [SC[:, 0:S], SC[:, 256:256 + S]]
                negd = [SC[:, 512:768], SC[:, 768:1024]]
                w0b, w1b = SC[:, 1024:1024 + S], SC[:, 1280:1280 + S]
                nc.gpsimd.partition_broadcast(w0b, sprt[:, 0, :])
                nc.gpsimd.partition_broadcast(w1b, sprt[:, 1, :])
                for mi, iot in ((0, io0), (1, io1)):
                    ma, mb = SC[:, 1536:1536 + S], SC[:, 1792:1792 + S]
                    ts_(nc.vector, ma, w0b, iot[:], None, AL.is_equal)
                    ts_(nc.vector, mb, w1b, iot[:], None, AL.is_equal)
                    nc.vector.tensor_max(ma, ma, mb)
                    ts_(nc.vector, negw[mi], ma, 1e9, 1e9, AL.mult, AL.subtract)
                for mi, rows in ((0, 128), (1, 127)):
                    ma, mb = SC[0:rows, 1536:1536 + L], SC[0:rows, 1792:1792 + L]
                    ts_(nc.vector, ma, iob[0:rows], spct[0:rows, mi, 0:1], None, AL.is_equal)
                    ts_(nc.vector, mb, iob[0:rows], spct[0:rows, mi, 1:2], None, AL.is_equal)
                    nc.vector.tensor_max(ma, ma, mb)
                    ts_(nc.vector, negd[mi][0:rows], ma, 1e9, 1e9, AL.mult, AL.subtract)
                labb = samp.tile([DEP, S], F32, tag="labb")
                nc.gpsimd.partition_broadcast(labb[:], sprt[:, 2, :], channels=DEP)
                eh = samp.tile([DEP, S], BF16, tag="eh")
                ts_(nc.vector, eh[:], labb[:], io0[0:DEP], None, AL.is_equal)

                # attention scratch (per sample): bcast rows + small cols
                SB = samp.tile([128, 1344], F32, tag="scrb")
                eldb, elwb = SB[:, 0:512], SB[:, 512:1024]

                def vrow_transpose(col_tiles, widths, dstap):
                    w = sum(widths)
                    rps = pvec.tile([1, 512], F32, tag="vrow")
                    off = 0
                    for ct, cw in zip(col_tiles, widths):
                        nc.tensor.matmul(rps[:, off:off + cw], ct, ident[0:cw, 0:cw],
                                         is_transpose=True, start=True, stop=True)
                        off += cw
                    row = SB[0:1, 1024:1024 + w]
                    nc.scalar.copy(row, rps[:, 0:w])
                    nc.gpsimd.partition_broadcast(dstap[:, 0:w], row)

                scn = [0]
                def attn_block(er_cols, el_bc, neg_tiles, rows_l, src_n, alpk):
                    als = []
                    for i, rows in enumerate(rows_l):
                        xb = attn.tile([128, 512], F32, tag="ax")
                        nc.vector.scalar_tensor_tensor(
                            xb[0:rows, 0:src_n], el_bc[0:rows, 0:src_n], er_cols[i],
                            neg_tiles[i][0:rows, 0:src_n], op0=AL.add, op1=AL.add)
                        nc.vector.scalar_tensor_tensor(
                            xb[0:rows, 0:src_n], xb[0:rows, 0:src_n], 0.2,
                            xb[0:rows, 0:src_n], op0=AL.mult, op1=AL.max)
                        cb = 1280 + 8 * ((scn[0]) % 8); scn[0] += 1
                        nmx = SB[0:rows, cb:cb + 1]
                        nc.vector.tensor_reduce(nmx, xb[0:rows, 0:src_n], AX.X, AL.max, negate=True)
                        p = attn.tile([128, 512], F32, tag="ap")
                        ssum = SB[0:rows, cb + 1:cb + 2]
                        nc.scalar.activation(p[0:rows, 0:src_n], xb[0:rows, 0:src_n], AF.Exp,
                                             bias=nmx, scale=1.0, accum_out=ssum)
                        r = SB[0:rows, cb + 2:cb + 3]
                        nc.vector.reciprocal(r, ssum)
                        gate = SB[0:rows, cb + 3:cb + 4]
                        ts_(nc.vector, gate, nmx, 1e7, None, AL.is_lt)
                        rg = SB[0:rows, cb + 4:cb + 5]
                        nc.vector.tensor_mul(rg, r, gate)
                        al = alpk[:, i, :]
                        ts_(nc.vector, al[0:rows, 0:src_n], p[0:rows, 0:src_n], rg, None, AL.mult)
                        als.append(al)
                    return als

                def alT_mm(al_tiles, dst_rows_l, src_n, place, width, tag):
                    src_tiles = []
                    n_src_t = (src_n + 127) // 128
                    for mi in range(n_src_t):
                        mw = min(128, src_n - 128 * mi)
                        ps = palt.tile([128, width], F32, tag=tag)
                        for ki, dr in enumerate(dst_rows_l):
                            nc.tensor.matmul(ps[0:mw, :], al_tiles[ki][0:dr, 128 * mi:128 * mi + mw],
                                             place[0:dr, ki, :], start=(ki == 0), stop=(ki == len(dst_rows_l) - 1))
                        sb = feat.tile([128, width], BF16, tag=tag + "s")
                        nc.scalar.copy(sb[0:mw, :], ps[0:mw, :])
                        src_tiles.append(sb)
                    return src_tiles

                # ---- layer 1 ----
                h1T = big.tile([128, K2, N], BF16, tag="h1T")
                for h in range(2):
                    colt = fet1.tile([128, 32], F32, tag="cols")
                    zsb, erw_c, elw_c = [], [], []
                    for m in range(2):
                        zps = pz.tile([128, 1024], F32, tag="z")
                        for n0, nw in ((0, 512), (512, 258)):
                            for k in range(KH):
                                nc.tensor.matmul(zps[:, n0:n0 + nw],
                                                 xts[:, k, 128 * m:128 * (m + 1)],
                                                 w1e[h][:, k, n0:n0 + nw],
                                                 start=(k == 0), stop=(k == KH - 1))
                        zb = feat.tile([128, H], BF16, tag="zw")
                        nc.scalar.copy(zb[:], zps[:, 0:H])
                        ec = colt[:, 16 * h + 2 * m:16 * h + 2 * m + 2]
                        nc.vector.tensor_copy(ec, zps[:, 768:770])
                        zsb.append(zb); elw_c.append(ec[:, 0:1]); erw_c.append(ec[:, 1:2])
                    zdsb, erd_c, eld_c = [], [], []
                    for m, rows in ((0, 128), (1, 127)):
                        zps = pz.tile([128, 1024], F32, tag="z")
                        for n0, nw in ((0, 512), (512, 258)):
                            nc.tensor.matmul(zps[0:rows, n0:n0 + nw],
                                             eh[:, 128 * m:128 * m + rows],
                                             ztab[h][:, n0:n0 + nw], start=True, stop=True)
                        zb = feat.tile([128, H], BF16, tag="zd")
                        nc.scalar.copy(zb[0:rows], zps[0:rows, 0:H])
                        ec = colt[:, 16 * h + 4 + 2 * m:16 * h + 4 + 2 * m + 2]
                        nc.vector.tensor_copy(ec[0:rows], zps[0:rows, 768:770])
                        zdsb.append(zb); eld_c.append(ec[0:rows, 0:1]); erd_c.append(ec[0:rows, 1:2])
                    vrow_transpose(eld_c, [128, 127], eldb)
                    vrow_transpose(elw_c, [128, 128], elwb)
                    alpW = attn.tile([128, 2, 256], BF16, tag="alw")
                    alpD = attn.tile([128, 2, 256], BF16, tag="ald")
                    alW = attn_block(erw_c, eldb, negw, [128, 128], S, alpW)
                    alD = attn_block(erd_c, elwb, negd, [128, 127], L, alpD)
                    aTW = alT_mm(alW, [128, 128], S, iwt, N, "atw")
                    aTD = alT_mm(alD, [128, 127], L, idt, N, "atd")
                    for wave in range(2):
                        pss = []
                        for mt in range(3 * wave, 3 * wave + 3):
                            ps = pagg.tile([128, N], F32, tag="agg")
                            first = True
                            for ks, rows in ((0, 128), (1, 127)):
                                nc.tensor.matmul(ps[:], zdsb[ks][0:rows, 128 * mt:128 * (mt + 1)],
                                                 aTW[ks][0:rows, :], start=first, stop=False)
                                first = False
                            for ks in (0, 1):
                                nc.tensor.matmul(ps[:], zsb[ks][:, 128 * mt:128 * (mt + 1)],
                                                 aTD[ks][:, :], start=False, stop=(ks == 1))
                            pss.append((mt, ps))
                        for mt, ps in pss:
                            ex = fet1.tile([128, N], F32, tag="elux")
                            nc.scalar.activation(ex[:], ps[:], AF.Exp)
                            ts_(nc.vector, ex[:], ex[:], 1.0, -1.0, AL.min, AL.add)
                            nc.vector.scalar_tensor_tensor(
                                h1T[:, KH * h + mt, :], ps[:], 0.0, ex[:], op0=AL.max, op1=AL.add)

                # ---- layer 2 ----
                colt2 = fet1.tile([128, 16], F32, tag="cols2")
                z2sb, er2_c, el2_c = [], [], []
                for m, rows in ((0, 128), (1, 128), (2, 128), (3, 127)):
                    zps = pz.tile([128, 1024], F32, tag="z")
                    for n0, nw in ((0, 512), (512, 258)):
                        for k in range(K2):
                            nc.tensor.matmul(zps[0:rows, n0:n0 + nw],
                                             h1T[:, k, 128 * m:128 * m + rows],
                                             w2e[:, k, n0:n0 + nw],
                                             start=(k == 0), stop=(k == K2 - 1))
                    zb = fet1.tile([128, H], BF16, tag=f"z2{m}")
                    nc.scalar.copy(zb[0:rows], zps[0:rows, 0:H])
                    ec = colt2[:, 2 * m:2 * m + 2]
                    nc.vector.tensor_copy(ec[0:rows], zps[0:rows, 768:770])
                    z2sb.append(zb); el2_c.append(ec[0:rows, 0:1]); er2_c.append(ec[0:rows, 1:2])
                vrow_transpose(el2_c[2:], [128, 127], eldb)
                vrow_transpose(el2_c[:2], [128, 128], elwb)
                alpW2 = attn.tile([128, 2, 256], BF16, tag="alw")
                alpD2 = attn.tile([128, 2, 256], BF16, tag="ald")
                alW2 = attn_block(er2_c[:2], eldb, negw, [128, 128], S, alpW2)
                alD2 = attn_block(er2_c[2:], elwb, negd, [128, 127], L, alpD2)
                aTW2 = alT_mm(alW2, [128, 128], S, jwt, NP, "atw")
                aTD2 = alT_mm(alD2, [128, 127], L, jdt, NP, "atd")
                gatT = big.tile([128, KH, NP], BF16, tag="gatT")
                for wave in range(2):
                    pss = []
                    for mt in range(3 * wave, 3 * wave + 3):
                        ps = pagg.tile([128, NP], F32, tag="agg")
                        first = True
                        for ks, rows in ((0, 128), (1, 127)):
                            nc.tensor.matmul(ps[:], z2sb[2 + ks][0:rows, 128 * mt:128 * (mt + 1)],
                                             aTW2[ks][0:rows, :], start=first, stop=False)
                            first = False
                        for ks in (0, 1):
                            nc.tensor.matmul(ps[:], z2sb[ks][:, 128 * mt:128 * (mt + 1)],
                                             aTD2[ks][:, :], start=False, stop=(ks == 1))
                        pss.append((mt, ps))
                    for mt, ps in pss:
                        nc.scalar.copy(gatT[:, mt, :], ps[:])

                # ---- xproj (both directions) ----
                for d in range(2):
                    for m in range(4):
                        pss = []
                        for ni in range(3):
                            ps = pagg.tile([128, 512], F32, tag="agg")
                            for k in range(KH):
                                nc.tensor.matmul(ps[:], gatT[:, k, 128 * m:128 * (m + 1)],
                                                 wihb[:, d, k, 512 * ni:512 * (ni + 1)],
                                                 start=(k == 0), stop=False)
                            nc.tensor.matmul(ps[:], onr[:, 128 * m:128 * (m + 1)],
                                             wihb[0:1, d, KH, 512 * ni:512 * (ni + 1)],
                                             start=False, stop=True)
                            pss.append((ni, ps))
                        xsb = big.tile([128, G4], F32R, tag=f"xsb{d}")
                        for ni, ps in pss:
                            nc.scalar.copy(xsb[:, 512 * ni:512 * (ni + 1)], ps[:])
                        nc.sync.dma_start(xp2[d, 128 * m:128 * (m + 1), s:s + 1, :],
                                          xsb[:].unsqueeze(1))

        gat_stack.close()  # free all GAT-phase weight tiles before the recurrence

        # ================= recurrence =================
        # state tiles persist across the hardware loop; updated in place.
        with tc.tile_pool(name="rst", bufs=1) as rst:
            whh2 = rst.tile([128, 2, 3, G4], F32R)
            nc.sync.dma_start(whh2[:], Whh[:])
            W3 = 3 * nsamp
            hT, cst = [], []
            zed = rst.tile([128, W3], F32)
            nc.vector.memset(zed[:], 0.0)
            for u in range(2):
                ht = rst.tile([128, W3], F32R, tag=f"hT{u}")
                nc.vector.tensor_copy(ht[:], zed[:])
                hT.append(ht)
                ct = rst.tile([nsamp, HH], F32, tag=f"c{u}")
                nc.vector.memset(ct[:], 0.0)
                cst.append(ct)

            with tc.tile_pool(name="rx", bufs=3) as rx, \
                 tc.tile_pool(name="rg", bufs=2) as rg, \
                 tc.tile_pool(name="pg", bufs=1, space="PSUM") as pgp, \
                 tc.tile_pool(name="ptr", bufs=1, space="PSUM") as ptr:

                def step(i):
                    for u in range(2):
                        xr = rx.tile([nsamp, G4], F32R, tag=f"xr{u}")
                        src = xp2[u][ds(i, 1)] if u == 0 else xp2[u][ds(nstep - 1 - i, 1)]
                        # NOTE: SBUF-side AP must keep the partition dim first
                        # ([nsamp, G4]); a leading unsqueeze would serialize all
                        # data into partition 0 and spill past the tile.
                        nc.sync.dma_start(xr[:], src)
                        if dbg is not None and "d_xr" in dbg:
                            nc.sync.dma_start(dbg["d_xr"][u], xr[:])
                        gps = pgp.tile([nsamp, G4], F32, tag=f"g{u}")
                        for nc0 in range(3):
                            o = gps[:, 512 * nc0:512 * (nc0 + 1)]
                            nc.tensor.matmul(o, i8r[:], xr[:, 512 * nc0:512 * (nc0 + 1)],
                                             start=True, stop=False)
                            for kc in range(3):
                                nc.tensor.matmul(o, hT[u][:, nsamp * kc:nsamp * (kc + 1)],
                                                 whh2[:, u, kc, 512 * nc0:512 * (nc0 + 1)],
                                                 start=False, stop=(kc == 2))
                        sg = rg.tile([nsamp, G3], F32, tag=f"sg{u}")
                        nc.scalar.activation(sg[:], gps[:, 0:G3], AF.Sigmoid)
                        tg = rg.tile([nsamp, HH], F32, tag=f"tg{u}")
                        nc.scalar.activation(tg[:], gps[:, G3:G4], AF.Tanh)
                        t1 = rg.tile([nsamp, HH], F32, tag=f"t1{u}")
                        nc.vector.tensor_mul(t1[:], sg[:, HH:2 * HH], cst[u][:])
                        t2 = rg.tile([nsamp, HH], F32, tag=f"t2{u}")
                        nc.vector.tensor_mul(t2[:], sg[:, 0:HH], tg[:])
                        nc.vector.tensor_add(cst[u][:], t1[:], t2[:])
                        th = rg.tile([nsamp, HH], F32, tag=f"th{u}")
                        nc.scalar.activation(th[:], cst[u][:], AF.Tanh)
                        hh = rg.tile([nsamp, HH], F32, tag=f"hh{u}")
                        nc.vector.tensor_mul(hh[:], sg[:, 2 * HH:G3], th[:])
                        tps = ptr.tile([128, W3], F32, tag=f"tr{u}")
                        for kc in range(3):
                            nc.tensor.matmul(tps[:, nsamp * kc:nsamp * (kc + 1)],
                                             hh[:, 128 * kc:128 * (kc + 1)],
                                             ident[0:nsamp, 0:nsamp],
                                             is_transpose=True, start=True, stop=True)
                        nc.vector.tensor_copy(hT[u][:], tps[:])

                tc.For_i_unrolled(0, nstep, 1, step, max_unroll=8)

            # ---- extract final hidden: out[s, :] = [h_fwd(s), h_bwd(s)] ----
            with tc.tile_pool(name="pfin", bufs=1, space="PSUM") as pfin:
                outt = rst.tile([nsamp, 2 * HH], F32)
                for u in range(2):
                    hf = rst.tile([128, W3], F32, tag=f"hf{u}")
                    nc.vector.tensor_copy(hf[:], hT[u][:])
                    fps = pfin.tile([nsamp, HH], F32, tag=f"f{u}")
                    for kc in range(3):
                        nc.tensor.matmul(fps[:, 128 * kc:128 * (kc + 1)],
                                         hf[:, nsamp * kc:nsamp * (kc + 1)], ident[:],
                                         is_transpose=True, start=True, stop=True)
                    nc.scalar.copy(outt[:, HH * u:HH * (u + 1)], fps[:])
                nc.sync.dma_start(out[:], outt[:])


# ===================== host-side preparation =====================
def _chunkP(a):
    """[X*128, ...] -> [128, X, ...] with p inner: out[p, c, ...] = a[128c+p]"""
    x = a.reshape(a.shape[0] // 128, 128, *a.shape[1:])
    return np.swapaxes(x, 0, 1)

def _bf(a):
    return np.ascontiguousarray(a.astype(ml_dtypes.bfloat16))

def _f(a):
    return np.ascontiguousarray(np.asarray(a, np.float32))

# torch gate order is (i, f, g, o); we reorder rows to (i, f, o, g) so the
# sigmoid gates are contiguous for one fused activation.
_GPERM = np.concatenate([np.arange(0, HH), np.arange(HH, 2 * HH),
                         np.arange(3 * HH, 4 * HH), np.arange(2 * HH, 3 * HH)])


def make_in_maps(inp, nsamp=NSAMP):
    emb = _f(inp["depend_emb"])
    in_maps = []
    embT_ = _bf(_chunkP(emb.T))                       # [128, KH, DEP]
    onehot = lambda idx, w: np.eye(w, dtype=np.float32)[idx]  # rows
    wid = np.arange(256)
    Iw_ = _bf(_chunkP(onehot(wid, N)))                # [128,2,N]
    sid = np.arange(255)
    Id_rows = np.zeros((256, N), np.float32); Id_rows[:255] = onehot(256 + sid, N)
    Id_ = _bf(_chunkP(Id_rows))
    Jw_ = _bf(_chunkP(onehot(wid, NP)))
    Jd_rows = np.zeros((256, NP), np.float32); Jd_rows[:255] = onehot(256 + sid, NP)
    Jd_ = _bf(_chunkP(Jd_rows))
    ones_ = _bf(np.ones((1, NP), np.float32))
    for c in range(8):
        g = "prem" if c < 4 else "hypo"
        q = c % 4
        lstm = "lstm1" if g == "prem" else "lstm2"
        sl = slice(nsamp * q, nsamp * (q + 1))
        hid = _f(inp[f"{g}_hidden_states"])[sl]       # [nsamp, L, H]
        spans = np.asarray(inp[f"{g}_span"])[sl]      # [nsamp, S, 3]
        m = {}
        m["xT"] = _bf(np.stack([_chunkP(hid[s].T) for s in range(nsamp)]))
        spc_ = np.zeros((nsamp, 256, 3), np.float32)
        spc_[:, :255] = spans.astype(np.float32)
        m["spc"] = _f(np.stack([_chunkP(spc_[s]) for s in range(nsamp)]))
        m["spr"] = _f(np.swapaxes(spans.astype(np.float32), 1, 2))
        m["embT"] = embT_
        W1 = _f(inp[f"{g}_W1"])                       # [2,H,H]
        m["W1"] = _bf(np.stack([_chunkP(W1[h]) for h in range(2)]))
        m["W1T"] = _bf(np.stack([_chunkP(W1[h].T) for h in range(2)], axis=1))
        a1 = _f(inp[f"{g}_a1"])                       # [2, 2H]
        m["a1"] = _bf(np.stack(
            [np.stack([_chunkP(a1[h, lr * H:(lr + 1) * H, None])[:, :, 0] for lr in range(2)])
             for h in range(2)]).transpose(2, 0, 1, 3))   # -> [128,2,2,KH]
        W2 = _f(inp[f"{g}_W2"])                       # [2H, H]
        m["W2"] = _bf(_chunkP(W2))
        m["W2T"] = _bf(_chunkP(W2.T))
        a2 = _f(inp[f"{g}_a2"])                       # [2H]
        m["a2"] = _bf(np.stack([_chunkP(a2[lr * H:(lr + 1) * H, None])[:, :, 0] for lr in range(2)])
                      .transpose(1, 0, 2))            # [128,2,KH]
        m["Iw"] = Iw_; m["Id"] = Id_
        m["Jw"] = Jw_; m["Jd"] = Jd_
        m["ones"] = ones_
        wihb_d, whh_d = [], []
        for dirn in ("f", "b"):
            Wih = _f(inp[f"{lstm}_Wih_{dirn}"])[_GPERM]   # [4HH, H] reordered
            bb = _f(inp[f"{lstm}_b_{dirn}"])[_GPERM]      # [4HH]
            Wihb_ = np.zeros((896, G4), np.float32)
            Wihb_[:H] = Wih.T
            Wihb_[H] = bb
            wihb_d.append(_chunkP(Wihb_))                 # [128, 7, G4]
            Whh_ = _f(inp[f"{lstm}_Whh_{dirn}"])[_GPERM]  # [4HH, HH]
            whh_d.append(_chunkP(Whh_.T))                 # [128, 3, G4]
        m["Wihb"] = _bf(np.stack(wihb_d, axis=1))         # [128, 2, 7, G4]
        m["Whh"] = _f(np.stack(whh_d, axis=1))            # [128, 2, 3, G4]
        in_maps.append(m)
    return in_maps


# ===================== harness entry point =====================
_NC_CACHE = {}

def _get_nc(nsamp=NSAMP, nstep=N):
    key = (nsamp, nstep)
    if key not in _NC_CACHE:
        _NC_CACHE[key] = build_nc(nsamp=nsamp, nstep=nstep)
    return _NC_CACHE[key]


def kernel(**inputs):
    """Full-input entry: shards across 8 NeuronCores, runs the Bass kernel,
    returns the full [32, 3] float32 output (bilinear combine on host)."""
    inputs = {k: np.asarray(v) for k, v in inputs.items()}
    nc = _get_nc()
    in_maps = make_in_maps(inputs, nsamp=NSAMP)
    from concourse import bass_utils
    res = bass_utils.run_bass_kernel_spmd(nc, in_maps, core_ids=list(range(8)))
    p_h = np.concatenate([res.results[q]["out"] for q in range(4)], 0)   # [32, 768]
    h_h = np.concatenate([res.results[4 + q]["out"] for q in range(4)], 0)
    bil_W = np.asarray(inputs["bil_W"], np.float32)
    bil_b = np.asarray(inputs["bil_b"], np.float32)
    out = np.einsum('bi,kij,bj->bk', p_h, bil_W, h_h) + bil_b
    return out.astype(np.float32)
